# revision 21
# baseline (speedup 1.0000x reference)
"""Trainium2 Bass kernel: AttentionBlock (GroupNorm + 1x1-conv QKV + MHA + proj + residual).

Data-parallel over batch: 16 samples -> 8 NeuronCores x 2 samples. Each core
runs the whole block locally (attention is per-sample, no collectives); the
host shards inputs and concatenates the 8 output shards.

Math notes (exact rewrites, not approximations):
  - scores are computed transposed, S^T[m,n] = sum_d k[d,m] q'[d,n] with
    q' = (q + b_q) * d^-0.5. The k-bias adds a column-constant to S^T which
    softmax cancels, so it is dropped.
  - softmax denominator comes from a ones-column appended to v^T in the
    attn@v matmul (row 64 of the [65, n] output accumulates colsum(exp S^T)).
  - v-bias: attn rows sum to 1, so  attn @ (Wv h + bv) = attn @ Wv h + bv;
    the bv term is folded into the proj bias on the host:
    beff = b_proj + w_proj @ bv.
"""

import os
import sys
import types
from contextlib import ExitStack

import ml_dtypes
import numpy as np

# If BASS_TRACE is set but this container's antenv lacks the NTFF hook
# module, bass_utils' trace path would crash on import; give it a null
# hook so tracing degrades gracefully instead.
try:
    import antenv.axon_hooks  # noqa: F401
except Exception:  # pragma: no cover
    try:
        import antenv

        _hookmod = types.ModuleType("antenv.axon_hooks")
        _hook = [None]
        _hookmod.set_axon_ntff_profile_hook = lambda h: _hook.__setitem__(0, h)
        _hookmod.get_axon_ntff_profile_hook = lambda: _hook[0]
        sys.modules["antenv.axon_hooks"] = _hookmod
        antenv.axon_hooks = _hookmod
    except Exception:
        pass

import concourse.bass as bass
import concourse.tile as tile
from concourse import bacc
from concourse import mybir
from concourse.bass_utils import run_bass_kernel_spmd

F32 = mybir.dt.float32
BF16 = mybir.dt.bfloat16
F8 = mybir.dt.float8e4
DR = mybir.MatmulPerfMode.DoubleRow
AF = mybir.ActivationFunctionType
ALU = mybir.AluOpType

# Problem dims (hardcoded per spec: x [16, 512, 32, 32] f32)
B, C, H, W = 16, 512, 32, 32
N = H * W                # 1024 spatial positions
NCORES = 8
BS = B // NCORES         # 2 samples per core
G = 32                   # groupnorm groups
HEADS = 8
D = C // HEADS           # 64
CT = C // 128            # 4 channel tiles
MT = N // 128            # 8 m-tiles (spatial, attention contraction)
NHALF = 2                # n split in halves of 512 (psum bank limit)
EPS = 1e-5
GROUP_ELEMS = (C // G) * N   # 16 ch * 1024 = 16384 per group

# Schraudolph fast-exp constants (fp8 bit trick): scores are computed
# pre-scaled by SCHS8 (folded into the q scale), so exp(s) is either
#   ACT: Exp activation with free-affine scale 1/SCHS8 + SHIFT bias (exact), or
#   DVE: int8(max(s' + SCHB8, 0)) reinterpreted as e4m3 (|rel err| <= ~6%),
# split across both engines to halve the softmax-exp critical path.
# All scores are globally shifted by SHIFT (softmax-invariant) so the max
# exp value (~e^5.3 = 198) stays below the e4m3 representable max and the
# Schraudolph int8 bits stay below the inf/nan encodings (>= 120).
SHIFT = -2.0
SCHS8 = 2.0 ** 3 / 0.6931471805599453  # 11.5416
SCHB8 = 7.0 * 8.0 - 0.344 + SHIFT * SCHS8  # mantissa-balanced fp8 bias + shift

LAST_EXEC_NS = None
LAST_RESULTS = None


def _build_tile(ctx: ExitStack, tc: tile.TileContext, te: dict):
    nc = tc.nc
    x_e, out_e = te["x"], te["out"]

    const = ctx.enter_context(tc.tile_pool(name="const", bufs=1))
    small = ctx.enter_context(tc.tile_pool(name="small", bufs=6))
    ps_acc = ctx.enter_context(tc.tile_pool(name="ps_acc", bufs=4, space="PSUM"))
    ps_sc = ctx.enter_context(tc.tile_pool(name="ps_sc", bufs=2, space="PSUM"))

    # Static bf16 tiles: junk operand for HAM-warming heater matmuls, and the
    # ones block whose col-packed matmul materializes softmax denominators on
    # psum partitions 64-127 (replacing the ones-column in vT + gpsimd
    # broadcast of the reciprocal).
    junk_sb = const.tile([128, 512], F8)
    nc.vector.memset(junk_sb[:], 0.0)
    # Static vT tile in the DoubleRow interleave [p, u, j, h, (vt|ones), d]:
    # contraction row m = (2u+j)*128 + p. Per head the stationary tile has 128
    # output columns: 0-63 the vT slice, 64-127 all-ones, so ONE DR matmul
    # yields av on psum rows 0-63 and the softmax denominators (replicated)
    # on rows 64-127 — DR requires full-array column tiling, so the ones ride
    # along as extra stationary columns instead of a second matmul.
    vt_sb = const.tile([128, MT // 2, 2, HEADS, 2, D], F8, name="vt_sb")
    nc.gpsimd.memset(vt_sb[:, :, :, :, 1, :], 1.0)

    # ---- groupnorm stats over the [BS*G, 16384] view of x ----
    # Issued before the weight loads: the stats chain gates the first matmul.
    # Chunked DMAs so bn_stats tracks the stream instead of one 4MB barrier.
    GCH = 4
    eps_sb = const.tile([BS * G, 1], F32)
    nc.vector.memset(eps_sb[:], EPS)
    shift_sb = const.tile([128, 1], F32)
    nc.vector.memset(shift_sb[:], SHIFT)
    # preload the Exp ACT table set off the critical path
    dummy_act = const.tile([1, 1], F32)
    nc.scalar.activation(dummy_act[:], eps_sb[0:1, :], AF.Exp)
    # stats on [128, 8192] half-group rows: full-width DMA ports (a [64, N]
    # layout would halve DMA bandwidth) and half the bn_stats calls
    NCHUNK = GROUP_ELEMS // 512
    stats_sb = const.tile([BS * G, NCHUNK, 6], F32)
    HSUB = GROUP_ELEMS // 2 // 512 // GCH  # 512-wide bn_stats per DMA chunk
    stats2 = const.tile([128, GROUP_ELEMS // 2 // 512, 6], F32)
    with tc.tile_pool(name="gnx", bufs=2) as gnxp:
        for gc in range(GCH):
            gnx = gnxp.tile([128, HSUB, 512], F32, tag="gnx", name="gnx")
            in_ap = bass.AP(
                tensor=x_e,
                offset=gc * HSUB * 512,
                ap=[[C * N, BS], [GROUP_ELEMS // 2, 2 * G], [1, HSUB * 512]],
            )
            nc.sync.dma_start(out=gnx[:], in_=in_ap)
            for j in range(HSUB):
                nc.vector.bn_stats(out=stats2[:, gc * HSUB + j, :], in_=gnx[:, j, :])
            # progressive fold of this chunk's half-group stats rows into the
            # [group, (half, chunk)] layout: the last fold lands right after
            # the last bn_stats instead of one big tail DMA.
            fold_dst = stats_sb[:].rearrange("g (h c) x -> g h c x", h=2)
            nc.sync.dma_start(
                out=fold_dst[:, :, gc * HSUB : (gc + 1) * HSUB, :],
                in_=stats2[:, gc * HSUB : (gc + 1) * HSUB, :],
            )

    # ---- PE heater: junk matmuls spanning the stats phase so the HAM clock
    # gate reaches (and holds) K=8/8 before the first real matmul, instead of
    # the PE idling ~30us and starting the qkv phase cold at 1.2 GHz.
    HEAT_MMS = 122
    heat_ps = ps_acc.tile([128, 512], F32, tag="acc", name="heat")
    for _ in range(HEAT_MMS):
        nc.tensor.matmul(
            heat_ps[:], junk_sb[:, 0:128], junk_sb[:], start=True, stop=True
        )

    # ---- constants / weights to SBUF (needed ~30us in; loads overlap stats).
    # The sel matrix is on the stats critical path, so it loads first. It
    # lives in the const pool: stack-reusing its space would make the x
    # loads wait for the sel matmuls (WAR on the reused bytes).
    sel_sb = const.tile([BS * G, BS * CT, 128], mybir.dt.float32r, name="sel_sb")
    nc.sync.dma_start(
        out=sel_sb[:], in_=te["sel"][:].rearrange("g (j p) -> g j p", p=128)
    )
    # fp8 weights in DoubleRow interleave layout [p, u, j, out]: contraction
    # row c = (2u+j)*128 + p; one DR matmul contracts 256 rows (u fixed,
    # j = the two interleaved layers).
    wqk_sb = const.tile([128, 2, 2, 2 * C], F8)
    wv_sb = const.tile([128, 2, 2, C], F8)
    wp_sb = const.tile([128, 2, 2, C], F8)
    bq_sb = const.tile([128, CT, 1], F32)
    beff_sb = const.tile([128, CT, 1], F32)
    nc.sync.dma_start(out=wqk_sb[:], in_=te["wqk8"][:])
    nc.sync.dma_start(out=wv_sb[:], in_=te["wv8"][:])
    for kt in range(CT):
        sl = slice(kt * 128, (kt + 1) * 128)
        nc.sync.dma_start(out=bq_sb[:, kt, :], in_=te["bq"][sl, :])
        nc.sync.dma_start(out=beff_sb[:, kt, :], in_=te["beff"][sl, :])
    nc.sync.dma_start(out=wp_sb[:], in_=te["wp8"][:])
    # gamma/beta replicated per sample: [128, (s, t)] layout
    gam2 = const.tile([128, BS * CT], F32)
    bet2 = const.tile([128, BS * CT], F32)
    for s in range(BS):
        nc.sync.dma_start(
            out=gam2[:, s * CT : (s + 1) * CT],
            in_=bass.AP(tensor=te["gamma"], offset=0, ap=[[1, 128], [128, CT]]),
        )
        nc.sync.dma_start(
            out=bet2[:, s * CT : (s + 1) * CT],
            in_=bass.AP(tensor=te["beta"], offset=0, ap=[[1, 128], [128, CT]]),
        )

    mv = small.tile([BS * G, 2], F32, tag="mv")
    nc.vector.bn_aggr(out=mv[:], in_=stats_sb[:])
    # rstd = rsqrt(var + eps) via int-seed + 2 Newton steps, all on DVE —
    # keeps ACT on the single Exp table set for the whole kernel (Ln/Sqrt
    # would force table reloads).
    I32 = mybir.dt.int32
    st2 = small.tile([BS * G, 2], mybir.dt.float32r, tag="st2")
    nc.vector.tensor_copy(st2[:, 0:1], mv[:, 0:1])
    vpe = small.tile([BS * G, 1], F32, tag="vpe")
    nc.vector.tensor_scalar_add(vpe[:], mv[:, 1:2], EPS)
    hv = small.tile([BS * G, 1], F32, tag="hv")
    nc.vector.tensor_scalar_mul(hv[:], vpe[:], -0.5)
    y0 = small.tile([BS * G, 1], F32, tag="y0")
    ysh = small.tile([BS * G, 1], I32, tag="ysh")
    nc.vector.tensor_scalar(
        out=ysh[:],
        in0=vpe[:].bitcast(I32),
        scalar1=1,
        scalar2=None,
        op0=ALU.arith_shift_right,
    )
    nc.vector.tensor_scalar(
        out=y0[:].bitcast(I32),
        in0=ysh[:],
        scalar1=-1,
        scalar2=0x5F3759DF,
        op0=ALU.mult,
        op1=ALU.add,
    )
    y1 = small.tile([BS * G, 1], F32, tag="y1")
    yw = small.tile([BS * G, 1], F32, tag="yw")
    # Newton: y <- y * (1.5 - 0.5*v*y^2), twice
    nc.vector.tensor_mul(yw[:], y0[:], y0[:])
    nc.vector.tensor_mul(yw[:], yw[:], hv[:])
    nc.vector.tensor_scalar_add(yw[:], yw[:], 1.5)
    nc.vector.tensor_mul(y1[:], y0[:], yw[:])
    nc.vector.tensor_mul(yw[:], y1[:], y1[:])
    nc.vector.tensor_mul(yw[:], yw[:], hv[:])
    nc.vector.tensor_scalar_add(yw[:], yw[:], 1.5)
    nc.vector.tensor_mul(st2[:, 1:2], y1[:], yw[:])

    # broadcast group stats to channel vectors with tiny selector matmuls on
    # the (otherwise idle) PE: mvr[p, j, :] = (mean, rstd) of group g(p, j);
    # SEL comes from the host, f32r keeps the stats at ~f32 precision
    A_all = const.tile([128, BS * CT], F32)
    B_all = const.tile([128, BS * CT], F32)
    mvr_ps = ps_sc.tile([128, BS * CT, 2], F32, tag="sc", name="mvr_ps")
    for j in range(BS * CT):
        nc.tensor.matmul(
            mvr_ps[:, j, :],
            sel_sb[:, j, :],
            st2[:],
            start=True,
            stop=True,
        )
    # h = x*A + Bv over all (s, t): A = rstd*gamma, Bv = beta - mean*A
    nc.vector.tensor_mul(A_all[:], mvr_ps[:, :, 1], gam2[:])
    tmpA = small.tile([128, BS * CT], F32, tag="tmpA")
    nc.vector.tensor_mul(tmpA[:], mvr_ps[:, :, 0], A_all[:])
    nc.vector.tensor_sub(B_all[:], bet2[:], tmpA[:])

    # Main pools open after the gn-stats/sel pools have freed their space.
    xpool = ctx.enter_context(tc.tile_pool(name="xbf", bufs=1))
    xrpool = ctx.enter_context(tc.tile_pool(name="xres", bufs=1))
    hpool = ctx.enter_context(tc.tile_pool(name="h", bufs=1))
    qkpool = ctx.enter_context(tc.tile_pool(name="qk", bufs=2))
    atpool = ctx.enter_context(tc.tile_pool(name="attn", bufs=2))
    aopool = ctx.enter_context(tc.tile_pool(name="ao", bufs=2))
    rbpool = ctx.enter_context(tc.tile_pool(name="rb", bufs=2))
    rcppool = ctx.enter_context(tc.tile_pool(name="rcps", bufs=2))
    outpool = ctx.enter_context(tc.tile_pool(name="outp", bufs=4))

    # h-apply input: bf16 copy of x (host-cast) — halves the startup DMA
    # bytes; h is quantized to fp8 right after, so the bf16 x loses nothing.
    # The f32 x for the residual streams just-in-time per sample instead.
    x_sb = xpool.tile([128, BS * CT, N], BF16)
    for s in range(BS):
        for t in range(CT):
            nc.gpsimd.dma_start(
                out=x_sb[:, s * CT + t, :],
                in_=te["xbf"][s, t * 128 : (t + 1) * 128, :],
            )

    def emit_xres(s):
        xres = xrpool.tile([128, CT, N], F32, tag="xres", name="xres")
        for t in range(CT):
            nc.gpsimd.dma_start(
                out=xres[:, t, :], in_=x_e[s, t * 128 : (t + 1) * 128, :]
            )
        return xres

    def emit_prep(s):
        # ---- groupnorm apply -> h (fp8), then qk / vT matmuls (DoubleRow) ----
        h_sb = hpool.tile([128, CT, N], F8, tag="h", name="h_sb")
        for t in range(CT):
            if t % 2 == 0:  # split h-apply across ACT and DVE
                nc.scalar.activation(
                    h_sb[:, t, :],
                    x_sb[:, s * CT + t, :],
                    AF.Identity,
                    bias=B_all[:, s * CT + t : s * CT + t + 1],
                    scale=A_all[:, s * CT + t : s * CT + t + 1],
                )
            else:
                nc.vector.tensor_scalar(
                    out=h_sb[:, t, :],
                    in0=x_sb[:, s * CT + t, :],
                    scalar1=A_all[:, s * CT + t : s * CT + t + 1],
                    scalar2=B_all[:, s * CT + t : s * CT + t + 1],
                    op0=ALU.mult,
                    op1=ALU.add,
                )
            if s == 0:
                # heater chained on the fresh h chunk: bridges the PE from the
                # static heater (ends with the stats stream) to the first
                # qkv matmuls without delaying them.
                nc.tensor.matmul(
                    heat_ps[:], h_sb[:, t, 0:128], h_sb[:, t, 0:512],
                    start=True, stop=True,
                )

        # ---- qk = wqkT.T @ h   ([o, n], o-tile p holds heads 2p, 2p+1) ----
        q_sb = qkpool.tile([128, CT, N], BF16, tag="q", name="q_sb")
        k_sb = qkpool.tile([128, CT, N], BF16, tag="k", name="k_sb")
        for o in range(2 * CT):
            for nh in range(NHALF):
                ps = ps_acc.tile([128, 512], F32, tag="acc", name="ps")
                for u in range(2):
                    nc.tensor.matmul(
                        ps[:],
                        wqk_sb[:, u, :, o * 128 : (o + 1) * 128],
                        h_sb[:, 2 * u : 2 * u + 2, nh * 512 : (nh + 1) * 512],
                        start=(u == 0),
                        stop=(u == 1),
                        perf_mode=DR,
                    )
                if o < CT:  # q: Schraudolph+attn scale and bias fused (ACT,
                    # bq_sb is pre-scaled on the host to match)
                    nc.scalar.activation(
                        q_sb[:, o, nh * 512 : (nh + 1) * 512],
                        ps[:],
                        AF.Identity,
                        bias=bq_sb[:, o, :],
                        scale=SCHS8 * float(D) ** -0.5,
                    )
                else:  # k channels: plain copy (bias dropped, see header)
                    nc.scalar.copy(
                        k_sb[:, o - CT, nh * 512 : (nh + 1) * 512], ps[:]
                    )

        # ---- vT = h.T @ wvT  ([m, dv]; fp8 into the static DR tile, ones
        # columns pre-set — see vt_sb comment) ----
        for m in range(MT):
            ps = ps_acc.tile([128, 512], F32, tag="acc", name="ps")
            for u in range(2):
                nc.tensor.matmul(
                    ps[:],
                    h_sb[:, 2 * u : 2 * u + 2, m * 128 : (m + 1) * 128],
                    wv_sb[:, u, :, :],
                    start=(u == 0),
                    stop=(u == 1),
                    perf_mode=DR,
                )
            nc.vector.tensor_copy(
                vt_sb[:, m // 2, m % 2, :, 0, :],
                ps[:].rearrange("p (h d) -> p h d", h=HEADS),
            )

        return q_sb, k_sb

    def emit_attention(s, q_sb, k_sb):
        # ---- attention: QK/exp of pair p interleaved with AV of pair p-1 ----
        # (fills the PE gaps while ACT runs exp; ~2x denser PE stream)
        ao_sb = aopool.tile([128, CT, N], F8, tag="ao", name="ao_sb")

        def emit_av_half(prev_state, u, hh):
            # one DoubleRow step for one head: contracts at m-chunks 2u and
            # 2u+1 at once; the stationary tile's ones columns put the
            # denominators on psum rows 64-127 of the same matmul for free.
            p0, at0, avs0 = prev_state
            for nh in range(NHALF):
                at_ap = at0[:, hh, 2 * u : 2 * u + 2, nh * 512 : (nh + 1) * 512]
                nc.tensor.matmul(
                    avs0[hh][nh][:, :],
                    vt_sb[:, u, :, 2 * p0 + hh, :, :],
                    at_ap,
                    start=(u == 0),
                    stop=(u == MT // 2 - 1),
                    perf_mode=DR,
                )

        def emit_normalize(prev_state, last=False):
            p0, at0, avs0 = prev_state
            # custom-DVE recip misreads PSUM sources on HW: SBUF-bounce the
            # (64-row replicated) denominator blocks via ACT, then one
            # full-width reciprocal per hh on DVE.
            rcps = []
            for hh in range(2):
                rb = rbpool.tile([64, N], F32, tag="rb", name="rb")
                for nh in range(NHALF):
                    nc.scalar.copy(
                        rb[:, nh * 512 : (nh + 1) * 512],
                        avs0[hh][nh][D : 2 * D, :],
                    )
                rcp = rcppool.tile([64, N], F32, tag="rcp", name="rcp")
                nc.vector.reciprocal_approx_fast(rcp[:], rb[:])
                rcps.append(rcp)
                if last:
                    # heater matmul chained on the freshly produced rcp:
                    # executes inside the drain-normalize window, keeping
                    # the HAM clock warm for the proj block.
                    nc.tensor.matmul(
                        heat_ps[:],
                        rcp[0:64, 0:128],
                        rcp[:, 0:512],
                        start=True,
                        stop=True,
                    )
            for nh in range(NHALF):  # nh-outer: proj closers need both hh of
                for hh in range(2):  # one nh first
                    nsl = slice(nh * 512, (nh + 1) * 512)
                    nc.vector.tensor_mul(
                        ao_sb[hh * 64 : (hh + 1) * 64, p0, nsl],
                        avs0[hh][nh][0:D, :],
                        rcps[hh][:, nsl],
                    )

        prev = None
        I8 = mybir.dt.int8
        for p in range(HEADS // 2):
            at_pair = atpool.tile([128, 2, MT, N], F8, tag="attn", name="at_pair")
            for m in range(MT):
                for hh in range(2):
                    base = hh * 64
                    sc = ps_sc.tile([128, N], F32, tag="sc", name="sc")
                    for nh in range(NHALF):
                        nsl = slice(nh * 512, (nh + 1) * 512)
                        nc.tensor.matmul(
                            sc[:, nsl],
                            k_sb[base : base + 64, p, m * 128 : (m + 1) * 128],
                            q_sb[base : base + 64, p, nsl],
                            start=True,
                            stop=True,
                            tile_position=(base, 0),
                        )
                    # softmax exp: each (m, hh) tile is split into its two
                    # nh halves, one per engine (ACT exact, DVE int8
                    # Schraudolph with a max-0 clamp so very negative scores
                    # can't wrap into the fp8 sign bit). Splitting halves the
                    # sc-psum hold time, which is the loop-carried critical
                    # path (QK(m+1) WARs on exp(m)). ACT takes both halves on
                    # two m's per pair to balance DVE's normalize load.
                    both_act = hh == 0 and m in (2, 5)
                    for nh in range(NHALF):
                        nsl = slice(nh * 512, (nh + 1) * 512)
                        if nh == 0 or both_act:
                            nc.scalar.activation(
                                at_pair[:, hh, m, nsl],
                                sc[:, nsl],
                                AF.Exp,
                                bias=shift_sb[:],
                                scale=1.0 / SCHS8,
                            )
                        else:
                            nc.vector.tensor_scalar(
                                out=at_pair[:, hh, m, nsl].bitcast(I8),
                                in0=sc[:, nsl],
                                scalar1=SCHB8,
                                scalar2=0.0,
                                op0=ALU.add,
                                op1=ALU.max,
                            )
                # spread the prev pair's AV-DR steps evenly over the m loop
                # (hh0 at odd m, hh1 at the next even m): a steady ~6
                # matmuls/m keeps the PE stream dense enough to hold the HAM
                # clock at full speed — bursty emission throttles it.
                if prev is not None:
                    if m % 2 == 1:
                        emit_av_half(prev, m // 2, 0)
                    elif m >= 2:
                        emit_av_half(prev, m // 2 - 1, 1)
            if prev is not None:
                emit_av_half(prev, MT // 2 - 1, 1)
            if prev is not None:
                emit_normalize(prev)
            if p < HEADS // 2 - 1:
                avs = [
                    [
                        ps_acc.tile([128, 512], F32, tag="acc", name=f"av{hh}_{nh}")
                        for nh in range(NHALF)
                    ]
                    for hh in range(2)
                ]
            else:
                # drain pair accumulates in the (then idle) scores pool so the
                # acc pool frees for the next sample's qkv before normalize
                dr = [
                    ps_sc.tile([128, N], F32, tag="sc", name=f"drain{hh}")
                    for hh in range(2)
                ]
                avs = [
                    [dr[hh][:, nh * 512 : (nh + 1) * 512] for nh in range(NHALF)]
                    for hh in range(2)
                ]
            prev = (p, at_pair, avs)
        # The drain-AV block waits for the last pair's trailing exps (its psum
        # tiles WAR on the final score tiles): bridge the PE hole with heater
        # matmuls chained on each trailing exp so they execute *inside* the
        # wait window and the clock gate stays at 8/8.
        for m in (5, 6, 6, 7, 7, 7):
            nc.tensor.matmul(
                heat_ps[:],
                junk_sb[:, 0:128],
                at_pair[:, m % 2, m, 0:512],
                start=True,
                stop=True,
            )
        for u in range(MT // 2):
            emit_av_half(prev, u, 0)
            emit_av_half(prev, u, 1)
        emit_normalize(prev, last=(s == BS - 1))
        return ao_sb

    def emit_proj(s, ao_sb, xres):
        # ---- proj + bias + residual, two waves of 4 open psum groups ----
        # u=0 partials need only pairs 0-1's ao, so they run while the
        # drain pair's normalize chain resolves; u=1 closes each group.
        for wave in range(4):
            pss = []
            for t in range(wave, wave + 1):
                for nh in range(NHALF):
                    nsl = slice(nh * 512, (nh + 1) * 512)
                    ps = ps_acc.tile(
                        [128, 512], F32, tag="acc", name=f"pj{t}_{nh}"
                    )
                    pss.append((t, nh, nsl, ps))
                    nc.tensor.matmul(
                        ps[:],
                        wp_sb[:, 0, :, t * 128 : (t + 1) * 128],
                        ao_sb[:, 0:2, nsl],
                        start=True,
                        stop=False,
                        perf_mode=DR,
                    )
            for t, nh, nsl, ps in pss:
                nc.tensor.matmul(
                    ps[:],
                    wp_sb[:, 1, :, t * 128 : (t + 1) * 128],
                    ao_sb[:, 2:4, nsl],
                    start=False,
                    stop=True,
                    perf_mode=DR,
                )
                ot = outpool.tile([128, 512], F32, tag="out", name="ot")
                nc.vector.scalar_tensor_tensor(
                    out=ot[:],
                    in0=ps[:],
                    scalar=beff_sb[:, t, :],
                    in1=xres[:, t, nsl],
                    op0=ALU.add,
                    op1=ALU.add,
                )
                nc.gpsimd.dma_start(
                    out=out_e[s, t * 128 : (t + 1) * 128, nsl], in_=ot[:]
                )

    # Drive: emit next sample's qkv prep between a sample's attention drain
    # and its proj, so the PE instruction stream has work while the
    # normalize (recip -> DRAM bounce -> broadcast) latency resolves.
    tiles = emit_prep(0)
    xres = emit_xres(0)
    for s in range(BS):
        ao = emit_attention(s, *tiles)
        if s + 1 < BS:
            tiles = emit_prep(s + 1)
        emit_proj(s, ao, xres)
        if s + 1 < BS:
            xres = emit_xres(s + 1)


def build_bass() -> bass.Bass:
    nc = bacc.Bacc()
    te = {
        "x": nc.declare_dram_parameter("x", [BS, C, N], F32, isOutput=False),
        "xbf": nc.declare_dram_parameter("xbf", [BS, C, N], BF16, isOutput=False),
        "wqk8": nc.declare_dram_parameter("wqk8", [128, 2, 2, 2 * C], F8, isOutput=False),
        "wv8": nc.declare_dram_parameter("wv8", [128, 2, 2, C], F8, isOutput=False),
        "wp8": nc.declare_dram_parameter("wp8", [128, 2, 2, C], F8, isOutput=False),
        "bq": nc.declare_dram_parameter("bq", [C, 1], F32, isOutput=False),
        "beff": nc.declare_dram_parameter("beff", [C, 1], F32, isOutput=False),
        "gamma": nc.declare_dram_parameter("gamma", [C, 1], F32, isOutput=False),
        "beta": nc.declare_dram_parameter("beta", [C, 1], F32, isOutput=False),
        "sel": nc.declare_dram_parameter(
            "sel", [BS * G, BS * CT * 128], mybir.dt.float32r, isOutput=False
        ),
        "out": nc.declare_dram_parameter("out", [BS, C, N], F32, isOutput=True),
    }
    with tile.TileContext(nc) as tc:
        with ExitStack() as ctx:
            _build_tile(ctx, tc, te)
    # Bacc defers register allocation to finalize(); run_bass_via_pjrt
    # serializes the module without calling it, so do it here.
    nc.finalize()
    return nc


def _make_sel() -> np.ndarray:
    sel = np.zeros((BS * G, BS * CT, 128), np.float32)
    for j in range(BS * CT):
        s0, t0 = j // CT, j % CT
        for p in range(128):
            sel[s0 * G + t0 * 8 + p // 16, j, p] = 1.0
    return sel.reshape(BS * G, BS * CT * 128)


def make_in_maps(inputs: dict) -> list[dict]:
    x = np.ascontiguousarray(np.asarray(inputs["x"], np.float32)).reshape(B, C, N)
    w_qkv = np.asarray(inputs["w_qkv"], np.float32)
    b_qkv = np.asarray(inputs["b_qkv"], np.float32)
    w_proj = np.asarray(inputs["w_proj"], np.float32)
    b_proj = np.asarray(inputs["b_proj"], np.float32)
    gamma = np.asarray(inputs["gamma"], np.float32)
    beta = np.asarray(inputs["beta"], np.float32)

    f8 = ml_dtypes.float8_e4m3

    def dr_layout(wT):
        # DoubleRow interleave: contraction row c = (2u+j)*128 + p -> [p,u,j,o]
        a = wT.reshape(2, 2, 128, wT.shape[1])
        return np.ascontiguousarray(a.transpose(2, 0, 1, 3)).astype(f8)

    common = {
        "wqk8": dr_layout(np.ascontiguousarray(w_qkv[: 2 * C, :].T)),
        "wv8": dr_layout(np.ascontiguousarray(w_qkv[2 * C :, :].T)),
        "wp8": dr_layout(np.ascontiguousarray(w_proj.T)),
        "bq": (b_qkv[:C] * (SCHS8 * float(D) ** -0.5)).reshape(C, 1).astype(np.float32),
        "beff": (b_proj + w_proj @ b_qkv[2 * C :]).reshape(C, 1).astype(np.float32),
        "gamma": gamma.reshape(C, 1).copy(),
        "beta": beta.reshape(C, 1).copy(),
        "sel": _make_sel(),
    }
    bf = ml_dtypes.bfloat16
    return [
        {
            "x": np.ascontiguousarray(x[i * BS : (i + 1) * BS]),
            "xbf": np.ascontiguousarray(x[i * BS : (i + 1) * BS]).astype(bf),
            **common,
        }
        for i in range(NCORES)
    ]


def kernel(**inputs) -> np.ndarray:
    global LAST_EXEC_NS, LAST_RESULTS
    nc = build_bass()
    in_maps = make_in_maps(inputs)
    res = run_bass_kernel_spmd(nc, in_maps, list(range(NCORES)))
    LAST_RESULTS = res
    LAST_EXEC_NS = res.exec_time_ns
    out = np.concatenate([np.asarray(res.results[i]["out"]) for i in range(NCORES)], 0)
    return out.reshape(B, C, H, W).astype(np.float32)

